# revision 22
# baseline (speedup 1.0000x reference)
"""Causal self-attention (B=2, L=2048, H=16, D=64) head-sharded over 8 TRN2 cores.

v3: software-pipelined single-pass program per core (local heads {2c, 2c+1}):
  - bf16 everywhere except psum accumulation, rowsums and rstd.
  - QKV^T projection from a resident x ring; A/B (rope halves) d-major,
    V computed token-major directly (no PE transposes). Each psum
    accumulation group is contiguous in the PE stream (HW requirement).
  - rmsnorm stats via one-hot matmuls; rstd = exp(-0.5*ln(var+eps)) and the
    activation-table list is reordered so Ln/Exp/Copy share one table (a
    single LoadActFuncSet for the whole program).
  - rope in bf16 on DVE/Pool; regather into per-head-contiguous Q^T/K^T via
    8 SBUF DMAs per 2 chunks.
  - attention: S^T per 128-k-block, exp with constant bias, diagonal-band
    blocks trimmed to their valid column span; P@V with appended ones-column
    for the softmax denominator; division broadcast via ones-matmuls.
  - emission interleaves b0 attention with b1 projection and b0 Wo with b1
    attention; PSUM rings: st 4 banks, aux 2, pv 2 (8 exactly).
  - Wo row-shard: each core emits a full-size bf16 partial out^T; host sums.
"""

import numpy as np
import ml_dtypes

import concourse.bacc as bacc
import concourse.bass as bass
import concourse.mybir as mybir
import concourse.tile as tile
from concourse import bass_utils
from concourse.hw_specs import get_activation_tables

F32 = mybir.dt.float32
F32R = mybir.dt.float32r
BF16 = mybir.dt.bfloat16

CFG = dict(B=2, L=2048, H=16, D=64, EPS=1e-6)
N_CORES = 8

TOKCH = 512   # token chunk for QKV projection
QCH = 512     # attention q chunk
KBLK = 128    # attention k block

ACT_TABLE = "natural_log_exp_and_others"


def _pin_act_table(nc):
    """Reorder the (cached) activation-table dict so the combined Ln+Exp+Copy
    table is picked first and the program needs a single table load."""
    try:
        tabs = get_activation_tables(nc.m.arch)
    except Exception:
        return
    if ACT_TABLE in tabs:
        mine = {mybir.ActivationFunctionType.Exp,
                mybir.ActivationFunctionType.Ln,
                mybir.ActivationFunctionType.Copy}
        for k in tabs:
            if k != ACT_TABLE:
                tabs[k] = tabs[k] - mine


def build_program(cfg, c_bias, debug=False):
    B, L, H, D = cfg["B"], cfg["L"], cfg["H"], cfg["D"]
    HID = H * D
    BT = B * L
    NHID = HID // 128           # hidden (contraction) chunks (8)
    NQC = L // QCH              # q chunks per batch (4)
    NKB = L // KBLK             # k blocks per batch (16)
    KPQ = QCH // KBLK           # k blocks per q chunk (4)
    CPB = L // TOKCH            # chunks per batch (4)
    scale = 1.0 / float(np.sqrt(D))
    Exp = mybir.ActivationFunctionType.Exp
    Ln = mybir.ActivationFunctionType.Ln

    nc = bacc.Bacc("TRN2", target_bir_lowering=False, debug=False,
                   num_devices=N_CORES)
    _pin_act_table(nc)

    xT = nc.dram_tensor("xT", [HID, BT], BF16, kind="ExternalInput").ap()
    wqkv = nc.dram_tensor("wqkv", [HID, 384], BF16, kind="ExternalInput").ap()
    wo = nc.dram_tensor("wo", [128, HID], BF16, kind="ExternalInput").ap()
    cs_d = nc.dram_tensor("cs_d", [128, BT], BF16, kind="ExternalInput").ap()
    sn_d = nc.dram_tensor("sn_d", [128, BT], BF16, kind="ExternalInput").ap()
    mask_d = nc.dram_tensor("mask_d", [128, 512], BF16, kind="ExternalInput").ap()
    sqind_d = nc.dram_tensor("sqind_d", [128, 4], BF16, kind="ExternalInput").ap()
    wA_d = nc.dram_tensor("wA_d", [4, 128], F32R, kind="ExternalInput").ap()
    wB_d = nc.dram_tensor("wB_d", [4, 128], F32R, kind="ExternalInput").ap()
    ones2_d = nc.dram_tensor("ones2_d", [2, 128], F32R, kind="ExternalInput").ap()
    outT = nc.dram_tensor("outT", [HID, BT], BF16, kind="ExternalOutput").ap()

    with tile.TileContext(nc) as tc:
        with tc.tile_pool(name="const", bufs=1) as const, \
             tc.tile_pool(name="big", bufs=1) as big, \
             tc.tile_pool(name="p1", bufs=2) as p1, \
             tc.tile_pool(name="stps", bufs=2, space="PSUM") as stps, \
             tc.tile_pool(name="auxps", bufs=2, space="PSUM") as auxps, \
             tc.tile_pool(name="pvps", bufs=1, space="PSUM") as pvps:

            # ---- constants / big tensors (DMAs emitted in schedule below)
            w_sb = const.tile([128, NHID, 384], BF16)
            cs_sb = const.tile([128, BT], BF16)
            sn_sb = const.tile([128, BT], BF16)
            wo_sb = const.tile([128, HID], BF16)
            mask_sb = const.tile([128, 512], BF16)
            sqind_sb = const.tile([128, 4], BF16)
            wA_sb = const.tile([4, 128], F32R)
            wB_sb = const.tile([4, 128], F32R)
            ones2_sb = const.tile([1, 256], F32R)
            eps_sb = const.tile([128, 1], F32)
            cb_sb = const.tile([128, 1], F32)

            QK = big.tile([128, 2 * BT], BF16)     # [:, 0:BT] = Q^T, [BT:] = K^T
            Vall = big.tile([128, NKB * B, 130], BF16)
            stage = [big.tile([128, L], BF16, name=f"stage{b}") for b in range(B)]
            attn_div = [big.tile([128, L], BF16, name=f"attn_div{b}")
                        for b in range(B)]
            h1tmp = big.tile([64, L], BF16)

            def emit_consts():
                nc.vector.memset(eps_sb, float(cfg["EPS"]))
                nc.vector.memset(cb_sb, -float(c_bias))
                nc.gpsimd.memset(Vall[:, :, 64:65], 1.0)
                nc.gpsimd.memset(Vall[:, :, 129:130], 1.0)
                nc.sync.dma_start(out=sqind_sb, in_=sqind_d)
                nc.sync.dma_start(out=wA_sb, in_=wA_d)
                nc.sync.dma_start(out=wB_sb, in_=wB_d)
                nc.sync.dma_start(out=ones2_sb, in_=ones2_d)

            # ---------- emission helpers ----------
            def emit_xload_split(t):      # one chunk, per-k DMAs (fast start)
                x_sb = p1.tile([128, NHID, 2 * TOKCH], BF16, tag="x", bufs=2,
                               name="x_sb")
                for k in range(NHID):
                    nc.sync.dma_start(
                        out=x_sb[:, k, 0:TOKCH],
                        in_=xT[128 * k:128 * (k + 1), t * TOKCH:(t + 1) * TOKCH])
                return x_sb

            def emit_xload_second(x_sb, t):   # fill second half of pair tile
                nc.sync.dma_start(
                    out=x_sb[:, :, TOKCH:2 * TOKCH],
                    in_=xT[:, t * TOKCH:(t + 1) * TOKCH].rearrange(
                        "(k p) t -> p k t", p=128))

            def emit_xload(pair):     # pair = t//2, loads 2 chunks of x
                x_sb = p1.tile([128, NHID, 2 * TOKCH], BF16, tag="x", bufs=2,
                               name="x_sb")
                span = slice(pair * 2 * TOKCH, (pair + 1) * 2 * TOKCH)
                nc.sync.dma_start(
                    out=x_sb,
                    in_=xT[:, span].rearrange("(k p) t -> p k t", p=128))
                return x_sb

            t1big = {}
            t2big = {}

            def emit_chunk(t, x_sb):
                xoff = (t % 2) * TOKCH
                xs = slice(xoff, xoff + TOKCH)
                blk0 = t * (TOKCH // KBLK)      # global 128-token block index
                # --- projection matmuls (each psum group contiguous)
                A_ps = auxps.tile([128, TOKCH], F32, tag="aux", name="A_ps")
                B_ps = auxps.tile([128, TOKCH], F32, tag="aux", name="B_ps")
                for m, ps in enumerate((A_ps, B_ps)):
                    for k in range(NHID):
                        nc.tensor.matmul(
                            ps, w_sb[:, k, 128 * m:128 * (m + 1)],
                            x_sb[:, k, xs],
                            start=(k == 0), stop=(k == NHID - 1))
                A_sb = p1.tile([128, TOKCH], BF16, tag="A", name="A_sb")
                B_sb = p1.tile([128, TOKCH], BF16, tag="B", name="B_sb")
                nc.vector.tensor_copy(A_sb, A_ps)
                nc.vector.tensor_copy(B_sb, B_ps)
                v_ps = auxps.tile([128, 4, 128], F32, tag="aux", name="v_ps")
                for blk in range(4):
                    for k in range(NHID):
                        nc.tensor.matmul(
                            v_ps[:, blk, :],
                            x_sb[:, k, xoff + 128 * blk: xoff + 128 * (blk + 1)],
                            w_sb[:, k, 256:384],
                            start=(k == 0), stop=(k == NHID - 1))
                nc.scalar.copy(
                    Vall[:, blk0:blk0 + 4, :].rearrange(
                        "p c (a d) -> p c a d", a=2)[:, :, :, 0:64],
                    v_ps.rearrange("p c (a d) -> p c a d", a=2))
                # --- rmsnorm stats
                sqA = p1.tile([128, TOKCH], BF16, tag="sqA", name="sqA")
                sqB = p1.tile([128, TOKCH], BF16, tag="sqB", name="sqB")
                nc.gpsimd.tensor_mul(sqA, A_sb, A_sb)
                nc.gpsimd.tensor_mul(sqB, B_sb, B_sb)
                var_ps = auxps.tile([4, TOKCH], F32, tag="aux", name="var_ps")
                nc.tensor.matmul(var_ps, sqind_sb, sqA, start=True, stop=False)
                nc.tensor.matmul(var_ps, sqind_sb, sqB, start=False, stop=True)
                lnv = p1.tile([4, TOKCH], F32, tag="lnv", name="lnv")
                nc.scalar.activation(lnv, var_ps, Ln, bias=eps_sb[0:4])
                rstd = p1.tile([4, TOKCH], F32R, tag="rstd", name="rstd")
                nc.scalar.activation(rstd, lnv, Exp, scale=-0.5)
                bcA_ps = auxps.tile([128, TOKCH], F32, tag="aux", name="bcA_ps")
                nc.tensor.matmul(bcA_ps, wA_sb, rstd, start=True, stop=True)
                An = p1.tile([128, TOKCH], BF16, tag="An", name="An")
                nc.vector.tensor_mul(An, A_sb, bcA_ps)
                bcB_ps = auxps.tile([128, TOKCH], F32, tag="aux", name="bcB_ps")
                nc.tensor.matmul(bcB_ps, wB_sb, rstd, start=True, stop=True)
                Bn = p1.tile([128, TOKCH], BF16, tag="Bn", name="Bn")
                nc.vector.tensor_mul(Bn, B_sb, bcB_ps)
                # --- rope (bf16)
                pair = t // 2
                if pair not in t1big:
                    t1big[pair] = p1.tile([128, 2 * TOKCH], BF16, tag="t1",
                                          bufs=2, name="t1big")
                    t2big[pair] = p1.tile([128, 2 * TOKCH], BF16, tag="t2",
                                          bufs=2, name="t2big")
                t1 = t1big[pair][:, xs]
                t2 = t2big[pair][:, xs]
                cs = cs_sb[:, t * TOKCH:(t + 1) * TOKCH]
                sn = sn_sb[:, t * TOKCH:(t + 1) * TOKCH]
                ta = p1.tile([128, TOKCH], BF16, tag="ta", name="ta")
                tb = p1.tile([128, TOKCH], BF16, tag="tb", name="tb")
                nc.vector.tensor_mul(ta, An, cs)
                nc.gpsimd.tensor_mul(tb, Bn, sn)
                nc.vector.tensor_sub(t1, ta, tb)
                tc_ = p1.tile([128, TOKCH], BF16, tag="tc", name="tc_")
                td = p1.tile([128, TOKCH], BF16, tag="td", name="td")
                nc.vector.tensor_mul(tc_, An, sn)
                nc.gpsimd.tensor_mul(td, Bn, cs)
                nc.vector.tensor_add(t2, tc_, td)
                # --- regather per pair of chunks
                if t % 2 == 1:
                    base = pair * 2 * TOKCH
                    for src, half in ((t1big[pair], 0), (t2big[pair], 1)):
                        for g in range(4):      # [q1h0|q1h1|k1h0|k1h1]
                            qk = g // 2         # 0 = q, 1 = k
                            h = g % 2
                            dst = QK[64 * h + 32 * half:64 * h + 32 * (half + 1),
                                     qk * BT + base: qk * BT + base + 2 * TOKCH]
                            nc.sync.dma_start(
                                out=dst, in_=src[32 * g:32 * (g + 1), :])

            def emit_att_j(b, j):
                nkb = KPQ * (j + 1)
                qbase = b * L + j * QCH
                pv = pvps.tile([65, 2, QCH], F32, tag="pv", name="pv")
                for i in range(nkb):
                    s_off = KBLK * i - QCH * j
                    diag = s_off >= 0
                    sp = slice(s_off, QCH) if diag else slice(0, QCH)
                    w = QCH - s_off if diag else QCH
                    st = stps.tile([128, 2, QCH], F32, tag="st", name="st")
                    for h in range(2):
                        nc.tensor.matmul(
                            st[:, h, sp],
                            QK[64 * h:64 * (h + 1),
                               BT + b * L + KBLK * i: BT + b * L + KBLK * (i + 1)],
                            QK[64 * h:64 * (h + 1), qbase + sp.start:qbase + QCH],
                            start=True, stop=True)
                    pexp = p1.tile([128, 2, QCH], BF16, tag="pexp", bufs=4,
                                   name="pexp")
                    nc.scalar.activation(
                        pexp[:, :, sp], st[:, :, sp],
                        Exp, bias=cb_sb, scale=scale)
                    if diag and w > 1:
                        nc.vector.tensor_mul(
                            pexp[:, 0, sp], pexp[:, 0, sp], mask_sb[:, 0:w])
                        nc.gpsimd.tensor_mul(
                            pexp[:, 1, sp], pexp[:, 1, sp], mask_sb[:, 0:w])
                    for h in range(2):
                        nc.tensor.matmul(
                            pv[:, h, sp],
                            Vall[:, b * NKB + i, 65 * h:65 * (h + 1)],
                            pexp[:, h, sp],
                            start=(i == 0), stop=diag,
                            skip_group_check=True)
                js = slice(j * QCH, (j + 1) * QCH)
                # rowsum reciprocal straight off psum row 64 (both heads)
                rsrow = p1.tile([1, 2, QCH], F32R, tag="rsrow", bufs=2,
                                name="rsrow")
                with nc.allow_low_precision(reason="f32r rowsum recip"):
                    nc.vector.reciprocal(rsrow, pv[64:65, :, :])
                # stage: h0 via DVE copy, h1 via DVE copy + DMA partition move
                nc.vector.tensor_copy(stage[b][0:64, js], pv[0:64, 0, :])
                nc.vector.tensor_copy(h1tmp[:, js], pv[0:64, 1, :])
                nc.sync.dma_start(out=stage[b][64:128, js], in_=h1tmp[:, js])
                # division: broadcast 1/rowsum per head via accumulating matmul
                bc = auxps.tile([128, QCH], F32, tag="aux", name="bc")
                nc.tensor.matmul(bc, ones2_sb[:, 0:128], rsrow[:, 0, :],
                                 start=True, stop=False)
                nc.tensor.matmul(bc, ones2_sb[:, 128:256], rsrow[:, 1, :],
                                 start=False, stop=True)
                nc.vector.tensor_mul(attn_div[b][:, js], stage[b][:, js], bc)

            def emit_wo(b, orange):           # o-major (attn_div fully ready)
                for o in orange:
                    ob = p1.tile([128, L], BF16, tag="ob", bufs=4, name="ob")
                    for jj in range(NQC):
                        js = slice(jj * QCH, (jj + 1) * QCH)
                        ops = auxps.tile([128, QCH], F32, tag="aux", name="ops")
                        nc.tensor.matmul(ops, wo_sb[:, 128 * o:128 * (o + 1)],
                                         attn_div[b][:, js],
                                         start=True, stop=True)
                        nc.vector.tensor_copy(ob[:, js], ops)
                    nc.sync.dma_start(
                        out=outT[128 * o:128 * (o + 1), b * L:(b + 1) * L],
                        in_=ob)

            ob1 = {}

            def emit_wo_cols(b, jj):          # jj-major (tail latency)
                js = slice(jj * QCH, (jj + 1) * QCH)
                for o in range(NHID):
                    if o not in ob1:
                        ob1[o] = p1.tile([128, L], BF16, tag="ob1", bufs=NHID,
                                         name="ob1")
                    ops = auxps.tile([128, QCH], F32, tag="aux", name="ops")
                    nc.tensor.matmul(ops, wo_sb[:, 128 * o:128 * (o + 1)],
                                     attn_div[b][:, js],
                                     start=True, stop=True)
                    if o % 2 == 0:
                        nc.vector.tensor_copy(ob1[o][:, js], ops)
                    else:
                        nc.scalar.copy(ob1[o][:, js], ops)
                if jj % 2 == 1:               # store a half per o
                    hs = slice((jj - 1) * QCH, (jj + 1) * QCH)
                    for o in range(NHID):
                        nc.sync.dma_start(
                            out=outT[128 * o:128 * (o + 1),
                                     b * L + hs.start:b * L + hs.stop],
                            in_=ob1[o][:, hs])

            # ---------- emission schedule ----------
            nc.sync.dma_start(out=w_sb, in_=wqkv.rearrange("(k p) c -> p k c",
                                                           p=128))
            x_cur = emit_xload_split(0)       # chunk 0 per-k for fast start
            emit_consts()
            # rope tables for the first two chunks ahead of their rope ops
            nc.sync.dma_start(out=cs_sb[:, 0:2 * TOKCH], in_=cs_d[:, 0:2 * TOKCH])
            nc.sync.dma_start(out=sn_sb[:, 0:2 * TOKCH], in_=sn_d[:, 0:2 * TOKCH])
            nc.sync.dma_start(out=mask_sb, in_=mask_d)
            emit_chunk(0, x_cur)
            emit_xload_second(x_cur, 1)
            emit_chunk(1, x_cur)
            x_cur = emit_xload(1)
            nc.sync.dma_start(out=cs_sb[:, 2 * TOKCH:], in_=cs_d[:, 2 * TOKCH:])
            nc.sync.dma_start(out=sn_sb[:, 2 * TOKCH:], in_=sn_d[:, 2 * TOKCH:])
            emit_chunk(2, x_cur)
            nc.sync.dma_start(out=wo_sb, in_=wo)
            emit_chunk(3, x_cur)
            for j in range(NQC):                      # b0 attention || b1 proj
                t = CPB + j
                if t % 2 == 0:
                    x_cur = emit_xload(t // 2)
                emit_chunk(t, x_cur)
                emit_att_j(0, j)
            for j in range(NQC):                      # b1 attention || b0 Wo
                emit_att_j(1, j)
                emit_wo(0, range(2 * j, 2 * j + 2))
                emit_wo_cols(1, j)
            if debug:
                dbg_qk = nc.dram_tensor("dbg_qk", [128, 2 * BT], BF16,
                                        kind="ExternalOutput").ap()
                dbg_vall = nc.dram_tensor("dbg_vall", [128, NKB * B * 130],
                                          BF16, kind="ExternalOutput").ap()
                dbg_stage = nc.dram_tensor("dbg_stage", [128, BT], BF16,
                                           kind="ExternalOutput").ap()
                dbg_ad = nc.dram_tensor("dbg_ad", [128, BT], BF16,
                                        kind="ExternalOutput").ap()
                nc.sync.dma_start(out=dbg_qk, in_=QK)
                nc.sync.dma_start(
                    out=dbg_vall,
                    in_=Vall.rearrange("p a b -> p (a b)"))
                for b in range(B):
                    nc.sync.dma_start(out=dbg_stage[:, b * L:(b + 1) * L],
                                      in_=stage[b])
                    nc.sync.dma_start(out=dbg_ad[:, b * L:(b + 1) * L],
                                      in_=attn_div[b])
    nc.compile()
    return nc


def prep_inputs(inputs, cfg):
    B, L, H, D = cfg["B"], cfg["L"], cfg["H"], cfg["D"]
    HID = H * D
    BT = B * L
    x = np.asarray(inputs["x"], np.float32)
    Wqkv = np.asarray(inputs["Wqkv"], np.float32)
    Wo = np.asarray(inputs["Wo"], np.float32)
    qw = np.asarray(inputs["q_norm_w"], np.float32)
    kw = np.asarray(inputs["k_norm_w"], np.float32)
    cos = np.asarray(inputs["cos"], np.float32)[:L]
    sin = np.asarray(inputs["sin"], np.float32)[:L]
    d2 = D // 2

    xT = np.ascontiguousarray(x.reshape(BT, HID).T).astype(ml_dtypes.bfloat16)
    # rope tables: rows grouped [q1h0|q1h1|k1h0|k1h1] each 32 = d2 dims,
    # columns = BT (batch-major), table indexed by l = tok % L
    ct = np.tile(cos.T, (4, B))                      # (128, BT)
    st_ = np.tile(sin.T, (4, B))
    cs_d = np.ascontiguousarray(ct).astype(ml_dtypes.bfloat16)
    sn_d = np.ascontiguousarray(st_).astype(ml_dtypes.bfloat16)
    ki = np.arange(128)[:, None]
    jj = np.arange(512)[None, :]
    mask_d = (jj >= ki).astype(ml_dtypes.bfloat16)
    sqind = np.zeros((128, 4), np.float32)
    sqind[np.arange(128), np.arange(128) // 32] = 1.0 / D
    sqind_d = sqind.astype(ml_dtypes.bfloat16)
    wA = np.zeros((4, 128), np.float32)
    wB = np.zeros((4, 128), np.float32)
    for m, w in enumerate([qw, qw, kw, kw]):
        cols = np.arange(32) + 32 * m
        wA[m, cols] = w[:d2]
        wB[m, cols] = w[d2:]
    ones2 = np.zeros((2, 128), np.float32)
    ones2[0, 0:64] = 1.0
    ones2[1, 64:128] = 1.0
    c_bias = float(np.sqrt(D) * max(np.abs(qw).max() * np.abs(kw).max(), 1e-6))

    hpc = H // N_CORES
    in_maps = []
    for c in range(N_CORES):
        h0 = hpc * c
        h1 = h0 + 1
        d32 = np.arange(d2)
        Acols = np.r_[h0 * D + d32, h1 * D + d32,
                      HID + h0 * D + d32, HID + h1 * D + d32]
        Bcols = Acols + d2
        Ccols = np.r_[2 * HID + h0 * D + np.arange(D),
                      2 * HID + h1 * D + np.arange(D)]
        w_c = np.ascontiguousarray(
            Wqkv[:, np.r_[Acols, Bcols, Ccols]]).astype(ml_dtypes.bfloat16)
        wo_c = np.ascontiguousarray(
            Wo[128 * c:128 * (c + 1), :]).astype(ml_dtypes.bfloat16)
        in_maps.append(dict(xT=xT, wqkv=w_c, wo=wo_c, cs_d=cs_d, sn_d=sn_d,
                            mask_d=mask_d, sqind_d=sqind_d,
                            wA_d=wA, wB_d=wB, ones2_d=ones2))
    return in_maps, c_bias


def gather_output(results, cfg):
    B, L, H, D = cfg["B"], cfg["L"], cfg["H"], cfg["D"]
    HID = H * D
    acc = np.zeros((HID, B * L), np.float32)
    for r in results:
        acc += r["outT"].astype(np.float32)
    return np.ascontiguousarray(acc.T).reshape(B, L, HID).astype(np.float32)


def kernel(**inputs):
    in_maps, c_bias = prep_inputs(inputs, CFG)
    nc = build_program(CFG, c_bias)
    res = bass_utils.run_bass_kernel_spmd(nc, in_maps, core_ids=list(range(N_CORES)))
    return gather_output(res.results, CFG)


# revision 23
# speedup vs baseline: 1.0056x; 1.0056x over previous
"""Causal self-attention (B=2, L=2048, H=16, D=64) head-sharded over 8 TRN2 cores.

v3: software-pipelined single-pass program per core (local heads {2c, 2c+1}):
  - bf16 everywhere except psum accumulation, rowsums and rstd.
  - QKV^T projection from a resident x ring; A/B (rope halves) d-major,
    V computed token-major directly (no PE transposes). Each psum
    accumulation group is contiguous in the PE stream (HW requirement).
  - rmsnorm stats via one-hot matmuls; rstd = exp(-0.5*ln(var+eps)) and the
    activation-table list is reordered so Ln/Exp/Copy share one table (a
    single LoadActFuncSet for the whole program).
  - rope in bf16 on DVE/Pool; regather into per-head-contiguous Q^T/K^T via
    8 SBUF DMAs per 2 chunks.
  - attention: S^T per 128-k-block, exp with constant bias, diagonal-band
    blocks trimmed to their valid column span; P@V with appended ones-column
    for the softmax denominator; division broadcast via ones-matmuls.
  - emission interleaves b0 attention with b1 projection and b0 Wo with b1
    attention; PSUM rings: st 4 banks, aux 2, pv 2 (8 exactly).
  - Wo row-shard: each core emits a full-size bf16 partial out^T; host sums.
"""

import numpy as np
import ml_dtypes

import concourse.bacc as bacc
import concourse.bass as bass
import concourse.mybir as mybir
import concourse.tile as tile
from concourse import bass_utils
from concourse.hw_specs import get_activation_tables

F32 = mybir.dt.float32
F32R = mybir.dt.float32r
BF16 = mybir.dt.bfloat16

CFG = dict(B=2, L=2048, H=16, D=64, EPS=1e-6)
N_CORES = 8

TOKCH = 512   # token chunk for QKV projection
QCH = 512     # attention q chunk
KBLK = 128    # attention k block

ACT_TABLE = "natural_log_exp_and_others"


def _pin_act_table(nc):
    """Reorder the (cached) activation-table dict so the combined Ln+Exp+Copy
    table is picked first and the program needs a single table load."""
    try:
        tabs = get_activation_tables(nc.m.arch)
    except Exception:
        return
    if ACT_TABLE in tabs:
        mine = {mybir.ActivationFunctionType.Exp,
                mybir.ActivationFunctionType.Ln,
                mybir.ActivationFunctionType.Copy}
        for k in tabs:
            if k != ACT_TABLE:
                tabs[k] = tabs[k] - mine


def build_program(cfg, c_bias, debug=False):
    B, L, H, D = cfg["B"], cfg["L"], cfg["H"], cfg["D"]
    HID = H * D
    BT = B * L
    NHID = HID // 128           # hidden (contraction) chunks (8)
    NQC = L // QCH              # q chunks per batch (4)
    NKB = L // KBLK             # k blocks per batch (16)
    KPQ = QCH // KBLK           # k blocks per q chunk (4)
    CPB = L // TOKCH            # chunks per batch (4)
    scale = 1.0 / float(np.sqrt(D))
    Exp = mybir.ActivationFunctionType.Exp
    Ln = mybir.ActivationFunctionType.Ln

    nc = bacc.Bacc("TRN2", target_bir_lowering=False, debug=False,
                   num_devices=N_CORES)
    _pin_act_table(nc)

    xT = nc.dram_tensor("xT", [HID, BT], BF16, kind="ExternalInput").ap()
    wqkv = nc.dram_tensor("wqkv", [HID, 384], BF16, kind="ExternalInput").ap()
    wo = nc.dram_tensor("wo", [128, HID], BF16, kind="ExternalInput").ap()
    cs_d = nc.dram_tensor("cs_d", [128, BT], BF16, kind="ExternalInput").ap()
    sn_d = nc.dram_tensor("sn_d", [128, BT], BF16, kind="ExternalInput").ap()
    mask_d = nc.dram_tensor("mask_d", [128, 512], BF16, kind="ExternalInput").ap()
    sqind_d = nc.dram_tensor("sqind_d", [128, 4], BF16, kind="ExternalInput").ap()
    wA_d = nc.dram_tensor("wA_d", [4, 128], F32R, kind="ExternalInput").ap()
    wB_d = nc.dram_tensor("wB_d", [4, 128], F32R, kind="ExternalInput").ap()
    ones2_d = nc.dram_tensor("ones2_d", [2, 128], F32R, kind="ExternalInput").ap()
    outT = nc.dram_tensor("outT", [HID, BT], BF16, kind="ExternalOutput").ap()

    with tile.TileContext(nc) as tc:
        with tc.tile_pool(name="const", bufs=1) as const, \
             tc.tile_pool(name="big", bufs=1) as big, \
             tc.tile_pool(name="p1", bufs=2) as p1, \
             tc.tile_pool(name="stps", bufs=2, space="PSUM") as stps, \
             tc.tile_pool(name="auxps", bufs=2, space="PSUM") as auxps, \
             tc.tile_pool(name="pvps", bufs=1, space="PSUM") as pvps:

            # ---- constants / big tensors (DMAs emitted in schedule below)
            w_sb = const.tile([128, NHID, 384], BF16)
            cs_sb = const.tile([128, BT], BF16)
            sn_sb = const.tile([128, BT], BF16)
            wo_sb = const.tile([128, HID], BF16)
            mask_sb = const.tile([128, 512], BF16)
            sqind_sb = const.tile([128, 4], BF16)
            wA_sb = const.tile([4, 128], F32R)
            wB_sb = const.tile([4, 128], F32R)
            ones2_sb = const.tile([1, 256], F32R)
            eps_sb = const.tile([128, 1], F32)
            cb_sb = const.tile([128, 1], F32)

            QK = big.tile([128, 2 * BT], BF16)     # [:, 0:BT] = Q^T, [BT:] = K^T
            Vall = big.tile([128, NKB * B, 130], BF16)
            stage = [big.tile([128, L], BF16, name=f"stage{b}") for b in range(B)]
            attn_div = [big.tile([128, L], BF16, name=f"attn_div{b}")
                        for b in range(B)]
            h1tmp = big.tile([64, L], BF16)

            def emit_consts():
                nc.vector.memset(eps_sb, float(cfg["EPS"]))
                nc.vector.memset(cb_sb, -float(c_bias))
                nc.gpsimd.memset(Vall[:, :, 64:65], 1.0)
                nc.gpsimd.memset(Vall[:, :, 129:130], 1.0)
                nc.sync.dma_start(out=sqind_sb, in_=sqind_d)
                nc.sync.dma_start(out=wA_sb, in_=wA_d)
                nc.sync.dma_start(out=wB_sb, in_=wB_d)
                nc.sync.dma_start(out=ones2_sb, in_=ones2_d)

            # ---------- emission helpers ----------
            def emit_xload_split(t):      # one chunk, per-k DMAs (fast start)
                x_sb = p1.tile([128, NHID, 2 * TOKCH], BF16, tag="x", bufs=2,
                               name="x_sb")
                for k in range(NHID):
                    nc.sync.dma_start(
                        out=x_sb[:, k, 0:TOKCH],
                        in_=xT[128 * k:128 * (k + 1), t * TOKCH:(t + 1) * TOKCH])
                return x_sb

            def emit_xload_second(x_sb, t):   # fill second half of pair tile
                nc.sync.dma_start(
                    out=x_sb[:, :, TOKCH:2 * TOKCH],
                    in_=xT[:, t * TOKCH:(t + 1) * TOKCH].rearrange(
                        "(k p) t -> p k t", p=128))

            def emit_xload(pair):     # pair = t//2, loads 2 chunks of x
                x_sb = p1.tile([128, NHID, 2 * TOKCH], BF16, tag="x", bufs=2,
                               name="x_sb")
                span = slice(pair * 2 * TOKCH, (pair + 1) * 2 * TOKCH)
                nc.sync.dma_start(
                    out=x_sb,
                    in_=xT[:, span].rearrange("(k p) t -> p k t", p=128))
                return x_sb

            t1big = {}
            t2big = {}

            def emit_chunk(t, x_sb):
                xoff = (t % 2) * TOKCH
                xs = slice(xoff, xoff + TOKCH)
                blk0 = t * (TOKCH // KBLK)      # global 128-token block index
                # --- projection matmuls (each psum group contiguous)
                A_ps = auxps.tile([128, TOKCH], F32, tag="aux", name="A_ps")
                B_ps = auxps.tile([128, TOKCH], F32, tag="aux", name="B_ps")
                for m, ps in enumerate((A_ps, B_ps)):
                    for k in range(NHID):
                        nc.tensor.matmul(
                            ps, w_sb[:, k, 128 * m:128 * (m + 1)],
                            x_sb[:, k, xs],
                            start=(k == 0), stop=(k == NHID - 1))
                A_sb = p1.tile([128, TOKCH], BF16, tag="A", name="A_sb")
                B_sb = p1.tile([128, TOKCH], BF16, tag="B", name="B_sb")
                nc.vector.tensor_copy(A_sb, A_ps)
                nc.vector.tensor_copy(B_sb, B_ps)
                v_ps = auxps.tile([128, 4, 128], F32, tag="aux", name="v_ps")
                for blk in range(4):
                    for k in range(NHID):
                        nc.tensor.matmul(
                            v_ps[:, blk, :],
                            x_sb[:, k, xoff + 128 * blk: xoff + 128 * (blk + 1)],
                            w_sb[:, k, 256:384],
                            start=(k == 0), stop=(k == NHID - 1))
                nc.scalar.copy(
                    Vall[:, blk0:blk0 + 4, :].rearrange(
                        "p c (a d) -> p c a d", a=2)[:, :, :, 0:64],
                    v_ps.rearrange("p c (a d) -> p c a d", a=2))
                # --- rmsnorm stats
                sqA = p1.tile([128, TOKCH], BF16, tag="sqA", name="sqA")
                sqB = p1.tile([128, TOKCH], BF16, tag="sqB", name="sqB")
                nc.gpsimd.tensor_mul(sqA, A_sb, A_sb)
                nc.gpsimd.tensor_mul(sqB, B_sb, B_sb)
                var_ps = auxps.tile([4, TOKCH], F32, tag="aux", name="var_ps")
                nc.tensor.matmul(var_ps, sqind_sb, sqA, start=True, stop=False)
                nc.tensor.matmul(var_ps, sqind_sb, sqB, start=False, stop=True)
                lnv = p1.tile([4, TOKCH], F32, tag="lnv", name="lnv")
                nc.scalar.activation(lnv, var_ps, Ln, bias=eps_sb[0:4])
                rstd = p1.tile([4, TOKCH], F32R, tag="rstd", name="rstd")
                nc.scalar.activation(rstd, lnv, Exp, scale=-0.5)
                bcA_ps = auxps.tile([128, TOKCH], F32, tag="aux", name="bcA_ps")
                nc.tensor.matmul(bcA_ps, wA_sb, rstd, start=True, stop=True)
                An = p1.tile([128, TOKCH], BF16, tag="An", name="An")
                nc.vector.tensor_mul(An, A_sb, bcA_ps)
                bcB_ps = auxps.tile([128, TOKCH], F32, tag="aux", name="bcB_ps")
                nc.tensor.matmul(bcB_ps, wB_sb, rstd, start=True, stop=True)
                Bn = p1.tile([128, TOKCH], BF16, tag="Bn", name="Bn")
                nc.vector.tensor_mul(Bn, B_sb, bcB_ps)
                # --- rope (bf16)
                pair = t // 2
                if pair not in t1big:
                    t1big[pair] = p1.tile([128, 2 * TOKCH], BF16, tag="t1",
                                          bufs=2, name="t1big")
                    t2big[pair] = p1.tile([128, 2 * TOKCH], BF16, tag="t2",
                                          bufs=2, name="t2big")
                t1 = t1big[pair][:, xs]
                t2 = t2big[pair][:, xs]
                cs = cs_sb[:, t * TOKCH:(t + 1) * TOKCH]
                sn = sn_sb[:, t * TOKCH:(t + 1) * TOKCH]
                ta = p1.tile([128, TOKCH], BF16, tag="ta", name="ta")
                tb = p1.tile([128, TOKCH], BF16, tag="tb", name="tb")
                nc.vector.tensor_mul(ta, An, cs)
                nc.gpsimd.tensor_mul(tb, Bn, sn)
                nc.vector.tensor_sub(t1, ta, tb)
                tc_ = p1.tile([128, TOKCH], BF16, tag="tc", name="tc_")
                td = p1.tile([128, TOKCH], BF16, tag="td", name="td")
                nc.vector.tensor_mul(tc_, An, sn)
                nc.gpsimd.tensor_mul(td, Bn, cs)
                nc.vector.tensor_add(t2, tc_, td)
                # --- regather per pair of chunks
                if t % 2 == 1:
                    base = pair * 2 * TOKCH
                    for src, half in ((t1big[pair], 0), (t2big[pair], 1)):
                        for g in range(4):      # [q1h0|q1h1|k1h0|k1h1]
                            qk = g // 2         # 0 = q, 1 = k
                            h = g % 2
                            dst = QK[64 * h + 32 * half:64 * h + 32 * (half + 1),
                                     qk * BT + base: qk * BT + base + 2 * TOKCH]
                            nc.sync.dma_start(
                                out=dst, in_=src[32 * g:32 * (g + 1), :])

            def emit_att_j(b, j):
                nkb = KPQ * (j + 1)
                qbase = b * L + j * QCH
                pv = pvps.tile([65, 2, QCH], F32, tag="pv", name="pv")
                for i in range(nkb):
                    s_off = KBLK * i - QCH * j
                    diag = s_off >= 0
                    sp = slice(s_off, QCH) if diag else slice(0, QCH)
                    w = QCH - s_off if diag else QCH
                    st = stps.tile([128, 2, QCH], F32, tag="st", name="st")
                    for h in range(2):
                        nc.tensor.matmul(
                            st[:, h, sp],
                            QK[64 * h:64 * (h + 1),
                               BT + b * L + KBLK * i: BT + b * L + KBLK * (i + 1)],
                            QK[64 * h:64 * (h + 1), qbase + sp.start:qbase + QCH],
                            start=True, stop=True)
                    pexp = p1.tile([128, 2, QCH], BF16, tag="pexp", bufs=4,
                                   name="pexp")
                    nc.scalar.activation(
                        pexp[:, :, sp], st[:, :, sp],
                        Exp, bias=cb_sb, scale=scale)
                    if diag and w > 1:
                        nc.vector.tensor_mul(
                            pexp[:, 0, sp], pexp[:, 0, sp], mask_sb[:, 0:w])
                        nc.gpsimd.tensor_mul(
                            pexp[:, 1, sp], pexp[:, 1, sp], mask_sb[:, 0:w])
                    for h in range(2):
                        nc.tensor.matmul(
                            pv[:, h, sp],
                            Vall[:, b * NKB + i, 65 * h:65 * (h + 1)],
                            pexp[:, h, sp],
                            start=(i == 0), stop=diag,
                            skip_group_check=True)
                js = slice(j * QCH, (j + 1) * QCH)
                # rowsum reciprocal straight off psum row 64 (both heads)
                rsrow = p1.tile([1, 2, QCH], F32R, tag="rsrow", bufs=2,
                                name="rsrow")
                with nc.allow_low_precision(reason="f32r rowsum recip"):
                    nc.vector.reciprocal(rsrow, pv[64:65, :, :])
                # stage: h0 via DVE copy, h1 via DVE copy + DMA partition move
                nc.vector.tensor_copy(stage[b][0:64, js], pv[0:64, 0, :])
                nc.vector.tensor_copy(h1tmp[:, js], pv[0:64, 1, :])
                nc.sync.dma_start(out=stage[b][64:128, js], in_=h1tmp[:, js])
                # division: broadcast 1/rowsum per head via accumulating matmul
                bc = auxps.tile([128, QCH], F32, tag="aux", name="bc")
                nc.tensor.matmul(bc, ones2_sb[:, 0:128], rsrow[:, 0, :],
                                 start=True, stop=False)
                nc.tensor.matmul(bc, ones2_sb[:, 128:256], rsrow[:, 1, :],
                                 start=False, stop=True)
                nc.vector.tensor_mul(attn_div[b][:, js], stage[b][:, js], bc)

            def emit_wo(b, orange):           # o-major (attn_div fully ready)
                for o in orange:
                    ob = p1.tile([128, L], BF16, tag="ob", bufs=4, name="ob")
                    for jj in range(NQC):
                        js = slice(jj * QCH, (jj + 1) * QCH)
                        ops = auxps.tile([128, QCH], F32, tag="aux", name="ops")
                        nc.tensor.matmul(ops, wo_sb[:, 128 * o:128 * (o + 1)],
                                         attn_div[b][:, js],
                                         start=True, stop=True)
                        nc.vector.tensor_copy(ob[:, js], ops)
                    nc.sync.dma_start(
                        out=outT[128 * o:128 * (o + 1), b * L:(b + 1) * L],
                        in_=ob)

            ob1 = {}

            def emit_wo_cols(b, jj):          # jj-major (tail latency)
                js = slice(jj * QCH, (jj + 1) * QCH)
                for o in range(NHID):
                    if o not in ob1:
                        ob1[o] = p1.tile([128, L], BF16, tag="ob1", bufs=NHID,
                                         name="ob1")
                    ops = auxps.tile([128, QCH], F32, tag="aux", name="ops")
                    nc.tensor.matmul(ops, wo_sb[:, 128 * o:128 * (o + 1)],
                                     attn_div[b][:, js],
                                     start=True, stop=True)
                    if o % 2 == 0:
                        nc.vector.tensor_copy(ob1[o][:, js], ops)
                    else:
                        nc.scalar.copy(ob1[o][:, js], ops)
                if jj % 2 == 1:               # store a half per o
                    hs = slice((jj - 1) * QCH, (jj + 1) * QCH)
                    for o in range(NHID):
                        nc.sync.dma_start(
                            out=outT[128 * o:128 * (o + 1),
                                     b * L + hs.start:b * L + hs.stop],
                            in_=ob1[o][:, hs])

            # ---------- emission schedule ----------
            nc.sync.dma_start(out=w_sb, in_=wqkv.rearrange("(k p) c -> p k c",
                                                           p=128))
            x_cur = emit_xload_split(0)       # chunk 0 per-k for fast start
            emit_consts()
            # rope tables for the first two chunks ahead of their rope ops
            nc.sync.dma_start(out=cs_sb[:, 0:2 * TOKCH], in_=cs_d[:, 0:2 * TOKCH])
            nc.sync.dma_start(out=sn_sb[:, 0:2 * TOKCH], in_=sn_d[:, 0:2 * TOKCH])
            nc.sync.dma_start(out=mask_sb, in_=mask_d)
            emit_chunk(0, x_cur)
            emit_xload_second(x_cur, 1)
            emit_chunk(1, x_cur)
            x_cur = emit_xload(1)
            nc.sync.dma_start(out=cs_sb[:, 2 * TOKCH:], in_=cs_d[:, 2 * TOKCH:])
            nc.sync.dma_start(out=sn_sb[:, 2 * TOKCH:], in_=sn_d[:, 2 * TOKCH:])
            emit_chunk(2, x_cur)
            nc.sync.dma_start(out=wo_sb, in_=wo)
            emit_chunk(3, x_cur)
            for j in range(NQC):                      # b0 attention || b1 proj
                emit_att_j(0, j)
                t = CPB + j
                if t % 2 == 0:
                    x_cur = emit_xload(t // 2)
                emit_chunk(t, x_cur)
            for j in range(NQC):                      # b1 attention || b0 Wo
                emit_att_j(1, j)
                emit_wo(0, range(2 * j, 2 * j + 2))
                emit_wo_cols(1, j)
            if debug:
                dbg_qk = nc.dram_tensor("dbg_qk", [128, 2 * BT], BF16,
                                        kind="ExternalOutput").ap()
                dbg_vall = nc.dram_tensor("dbg_vall", [128, NKB * B * 130],
                                          BF16, kind="ExternalOutput").ap()
                dbg_stage = nc.dram_tensor("dbg_stage", [128, BT], BF16,
                                           kind="ExternalOutput").ap()
                dbg_ad = nc.dram_tensor("dbg_ad", [128, BT], BF16,
                                        kind="ExternalOutput").ap()
                nc.sync.dma_start(out=dbg_qk, in_=QK)
                nc.sync.dma_start(
                    out=dbg_vall,
                    in_=Vall.rearrange("p a b -> p (a b)"))
                for b in range(B):
                    nc.sync.dma_start(out=dbg_stage[:, b * L:(b + 1) * L],
                                      in_=stage[b])
                    nc.sync.dma_start(out=dbg_ad[:, b * L:(b + 1) * L],
                                      in_=attn_div[b])
    nc.compile()
    return nc


def prep_inputs(inputs, cfg):
    B, L, H, D = cfg["B"], cfg["L"], cfg["H"], cfg["D"]
    HID = H * D
    BT = B * L
    x = np.asarray(inputs["x"], np.float32)
    Wqkv = np.asarray(inputs["Wqkv"], np.float32)
    Wo = np.asarray(inputs["Wo"], np.float32)
    qw = np.asarray(inputs["q_norm_w"], np.float32)
    kw = np.asarray(inputs["k_norm_w"], np.float32)
    cos = np.asarray(inputs["cos"], np.float32)[:L]
    sin = np.asarray(inputs["sin"], np.float32)[:L]
    d2 = D // 2

    xT = np.ascontiguousarray(x.reshape(BT, HID).T).astype(ml_dtypes.bfloat16)
    # rope tables: rows grouped [q1h0|q1h1|k1h0|k1h1] each 32 = d2 dims,
    # columns = BT (batch-major), table indexed by l = tok % L
    ct = np.tile(cos.T, (4, B))                      # (128, BT)
    st_ = np.tile(sin.T, (4, B))
    cs_d = np.ascontiguousarray(ct).astype(ml_dtypes.bfloat16)
    sn_d = np.ascontiguousarray(st_).astype(ml_dtypes.bfloat16)
    ki = np.arange(128)[:, None]
    jj = np.arange(512)[None, :]
    mask_d = (jj >= ki).astype(ml_dtypes.bfloat16)
    sqind = np.zeros((128, 4), np.float32)
    sqind[np.arange(128), np.arange(128) // 32] = 1.0 / D
    sqind_d = sqind.astype(ml_dtypes.bfloat16)
    wA = np.zeros((4, 128), np.float32)
    wB = np.zeros((4, 128), np.float32)
    for m, w in enumerate([qw, qw, kw, kw]):
        cols = np.arange(32) + 32 * m
        wA[m, cols] = w[:d2]
        wB[m, cols] = w[d2:]
    ones2 = np.zeros((2, 128), np.float32)
    ones2[0, 0:64] = 1.0
    ones2[1, 64:128] = 1.0
    c_bias = float(np.sqrt(D) * max(np.abs(qw).max() * np.abs(kw).max(), 1e-6))

    hpc = H // N_CORES
    in_maps = []
    for c in range(N_CORES):
        h0 = hpc * c
        h1 = h0 + 1
        d32 = np.arange(d2)
        Acols = np.r_[h0 * D + d32, h1 * D + d32,
                      HID + h0 * D + d32, HID + h1 * D + d32]
        Bcols = Acols + d2
        Ccols = np.r_[2 * HID + h0 * D + np.arange(D),
                      2 * HID + h1 * D + np.arange(D)]
        w_c = np.ascontiguousarray(
            Wqkv[:, np.r_[Acols, Bcols, Ccols]]).astype(ml_dtypes.bfloat16)
        wo_c = np.ascontiguousarray(
            Wo[128 * c:128 * (c + 1), :]).astype(ml_dtypes.bfloat16)
        in_maps.append(dict(xT=xT, wqkv=w_c, wo=wo_c, cs_d=cs_d, sn_d=sn_d,
                            mask_d=mask_d, sqind_d=sqind_d,
                            wA_d=wA, wB_d=wB, ones2_d=ones2))
    return in_maps, c_bias


def gather_output(results, cfg):
    B, L, H, D = cfg["B"], cfg["L"], cfg["H"], cfg["D"]
    HID = H * D
    acc = np.zeros((HID, B * L), np.float32)
    for r in results:
        acc += r["outT"].astype(np.float32)
    return np.ascontiguousarray(acc.T).reshape(B, L, HID).astype(np.float32)


def kernel(**inputs):
    in_maps, c_bias = prep_inputs(inputs, CFG)
    nc = build_program(CFG, c_bias)
    res = bass_utils.run_bass_kernel_spmd(nc, in_maps, core_ids=list(range(N_CORES)))
    return gather_output(res.results, CFG)
